# revision 1
# baseline (speedup 1.0000x reference)
"""Trainium2 Bass kernel for nn_JSDPosLoss: JSD loss over top-k retrieved rows.

Contract: kernel(**inputs) takes FULL numpy inputs, returns FULL output (f32 scalar).
Data-parallel over batch across 8 NeuronCores (4 batches/core).

Host prep (sharding): gathers sample_z / sample_z_dis (rand_idx is a host-known
input), transposes z_pos to (b, d, hw) so the device streams contraction-major
tiles directly, builds the JSD P matrix (pure broadcast of sample_z_dis).

Per-core device work:
  - stream z_posT tiles (16 MiB, the memory-bound part) across 3 DMA queues
  - matmul vs pre-gathered sample_z^T -> attn rows (batch bi at partitions
    32*bi..32*bi+2; compute-engine SBUF access must start at 0/32/64/96)
  - top-10 per (batch, query) row via DVE max8/max_index/match_replace
  - indirect-DMA gather of selected z_pos_dis rows (120 rows x 512)
  - JSD elementwise terms + free-dim reduction -> (120, 1) partial sums
Host: final scalar reduce + scale.
"""

import numpy as np

import concourse.bass as bass
import concourse.bacc as bacc
import concourse.mybir as mybir
import concourse.tile as tile
from concourse.bass_utils import run_bass_kernel_spmd

# Problem dims (hardcoded per contract)
B, H, W, D, NPQ = 32, 64, 64, 256, 512
HW = H * W                  # 4096
NQ, NPOS = 3, 10
NCORES = 8
BPC = B // NCORES           # 4 batches per core
NROW = BPC * NQ             # 12 attention rows per core
NPAIR = BPC * NQ * NPOS     # 120 JSD pair-rows per core

F32 = mybir.dt.float32
F32R = mybir.dt.float32r
U32 = mybir.dt.uint32

JH = 2048                   # j-columns per load (1 MiB per DMA)
MM_DTYPE = F32R            # matmul input dtype (F32 exact / F32R relaxed)


def build_kernel():
    nc = bacc.Bacc("TRN2", target_bir_lowering=False, debug=False,
                   num_devices=NCORES)

    # z_pos transposed on host: (BPC, 2, 128, HW), [bi, ck, cl, j]
    zpt = nc.dram_tensor("zpt", [BPC, 2, 128, HW], MM_DTYPE,
                         kind="ExternalInput").ap()
    zpdis = nc.dram_tensor("zpdis", [BPC * HW, NPQ], F32,
                           kind="ExternalInput").ap()
    szt = nc.dram_tensor("szt", [2, 128, 128], MM_DTYPE,
                         kind="ExternalInput").ap()
    pmat = nc.dram_tensor("pmat", [NPAIR, NPQ], F32, kind="ExternalInput").ap()
    boffs = nc.dram_tensor("boffs", [128, 1], F32, kind="ExternalInput").ap()
    out = nc.dram_tensor("out", [NPAIR, 1], F32, kind="ExternalOutput").ap()

    with tile.TileContext(nc) as tc:
        _body(tc, nc, zpt, zpdis, szt, pmat, boffs, out)
    nc.compile()
    return nc


def _body(tc, nc, zpt, zpdis, szt, pmat, boffs, out):
    NJQ = 4                     # j-quarters; topk rounds stream per quarter
    JQ = HW // NJQ              # 1024
    with (
        tc.tile_pool(name="const", bufs=1) as cpool,
        tc.tile_pool(name="load", bufs=6) as lpool,
        tc.tile_pool(name="atp", bufs=4, space="PSUM") as atp_pool,
        tc.tile_pool(name="qtk", bufs=2) as qpool,
        tc.tile_pool(name="small", bufs=1) as spool,
        tc.tile_pool(name="jsd", bufs=1) as jpool,
    ):
        # sample_z^T padded to 32 columns per batch (queries at 32*bi+q,
        # zeros elsewhere): matmuls then initialize all 128 attn partitions
        # (MM_DTYPE tiles: the DMA rounds f32 -> f32r at the producer, as the
        # BIR verifier requires for fp32r matmul operands)
        szt_sb = cpool.tile([128, 256], MM_DTYPE)
        nc.sync.dma_start(szt_sb[:, 0:128], szt[0])
        nc.sync.dma_start(szt_sb[:, 128:256], szt[1])

        # constants / independent loads, issued early
        boff = spool.tile([128, 1], F32)
        nc.sync.dma_start(boff[:], boffs[:, :])
        pm = jpool.tile([NPAIR, NPQ], F32)
        nc.scalar.dma_start(pm[:], pmat[:, :])

        # attention rows in SBUF: batch bi at partitions 32*bi..32*bi+2
        # (fp32r matmuls may only write PSUM at partition base 0, so each
        # (3, 512) slice lands in a partition-0 PSUM tile and DVE moves it)
        attn = cpool.tile([128, HW], F32)

        # per-quarter candidate maxima (top-16 per quarter per row)
        cand = cpool.tile([128, NJQ * 16], F32)

        # DMA issue queues: SP + ACT (HWDGE) + Pool (SWDGE), round-robin
        dma_engines = [nc.sync, nc.gpsimd, nc.scalar, nc.sync, nc.gpsimd]
        qi = 0

        for jq in range(NJQ):
            for bi in range(BPC):
                lds = []
                for ck in range(2):
                    ld = lpool.tile([128, JQ], MM_DTYPE, tag=f"ld{ck}")
                    eng = dma_engines[qi % len(dma_engines)]
                    qi += 1
                    eng.dma_start(ld[:], zpt[bi, ck, :, jq * JQ:(jq + 1) * JQ])
                    lds.append(ld)
                at_ps = atp_pool.tile([32, JQ], F32, tag="at_ps")
                for js in range(JQ // 512):
                    for ck in range(2):
                        nc.tensor.matmul(
                            at_ps[:, js * 512:(js + 1) * 512],
                            lhsT=szt_sb[:, ck * 128 + 32 * bi:
                                        ck * 128 + 32 * bi + 32],
                            rhs=lds[ck][:, js * 512:(js + 1) * 512],
                            start=(ck == 0), stop=(ck == 1))
                # one ACT copy per (batch, quarter); M=32 with zero-padded
                # queries also initializes the garbage attn partitions
                nc.scalar.copy(
                    attn[32 * bi:32 * bi + 32, jq * JQ:(jq + 1) * JQ],
                    at_ps[:])
            # streamed topk round for this quarter: top-16 values per row
            aq = attn[:, jq * JQ:(jq + 1) * JQ]
            c0 = cand[:, jq * 16:jq * 16 + 8]
            c1 = cand[:, jq * 16 + 8:jq * 16 + 16]
            nc.vector.max(c0, aq)
            tmpq = qpool.tile([128, JQ], F32, tag="tmpq")
            nc.vector.match_replace(tmpq[:], in_to_replace=c0,
                                    in_values=aq, imm_value=-1e30)
            nc.vector.max(c1, tmpq[:])

        # ---- merge quarters: top-10 values per row out of 64 candidates ----
        mv1 = spool.tile([128, 8], F32)
        nc.vector.max(mv1[:], cand[:])
        cand2 = spool.tile([128, NJQ * 16], F32)
        nc.vector.match_replace(cand2[:], in_to_replace=mv1[:],
                                in_values=cand[:], imm_value=-1e30)
        mv2 = spool.tile([128, 8], F32)
        nc.vector.max(mv2[:], cand2[:])
        mv10 = spool.tile([128, NPOS], F32)
        nc.vector.tensor_copy(mv10[:, 0:8], mv1[:])
        nc.vector.tensor_copy(mv10[:, 8:NPOS], mv2[:, 0:2])

        # ---- resolve indices: two max_index scans of the full attn row ----
        ix1 = spool.tile([128, 8], U32)
        ix2 = spool.tile([128, 8], U32)
        nc.vector.max_index(ix1[:], mv10[:, 0:8], attn[:])
        nc.vector.max_index(ix2[:], mv10[:, 2:NPOS], attn[:])

        idx10 = spool.tile([128, NPOS], U32)
        nc.vector.tensor_copy(idx10[:, 0:8], ix1[:])
        nc.vector.tensor_copy(idx10[:, 8:NPOS], ix2[:, 6:8])

        # add bi*HW so indices address the flattened (BPC*HW, NPQ) table
        # (f32 arithmetic: indices < 16384 are exact; cast back to u32 after)
        idx10f = spool.tile([128, NPOS], F32)
        nc.vector.tensor_copy(idx10f[:], idx10[:])
        nc.vector.tensor_scalar(idx10f[:], idx10f[:], boff[:], None,
                                op0=mybir.AluOpType.add)
        nc.vector.tensor_copy(idx10[:], idx10f[:])

        # flatten the 12 valid rows -> (120, 1); order (bi, q, k)
        # (spread across queues so the tiny DMAs overlap)
        idx_flat = spool.tile([NPAIR, 1], U32)
        for bi, eng in zip(range(BPC),
                           (nc.sync, nc.scalar, nc.gpsimd, nc.sync)):
            eng.dma_start(idx_flat[30 * bi:30 * (bi + 1), :],
                          idx10[32 * bi:32 * bi + NQ, :])

        # ---- gather the selected z_pos_dis rows ----
        gmat = jpool.tile([NPAIR, NPQ], F32)
        nc.gpsimd.indirect_dma_start(
            out=gmat[:], out_offset=None,
            in_=zpdis[:, :],
            in_offset=bass.IndirectOffsetOnAxis(ap=idx_flat[:, :1], axis=0))

        # ---- JSD terms: xlogy(p,p) + xlogy(g,g) - (p+g)*log(clip((p+g)/2)) ----
        # Ln(x*scale + bias) fusion on ACT: bias 1e-7/1e-38 stands in for the
        # clip/xlogy(0,0) guards (error <= ~1e-6 relative, values in [0, 1))
        bias7 = jpool.tile([NPAIR, 1], F32)
        nc.vector.memset(bias7[:], 1e-7)
        bias38 = jpool.tile([NPAIR, 1], F32)
        nc.vector.memset(bias38[:], 1e-38)

        s = jpool.tile([NPAIR, NPQ], F32)
        nc.vector.tensor_add(s[:], pm[:], gmat[:])
        m = jpool.tile([NPAIR, NPQ], F32)
        nc.scalar.activation(m[:], s[:], mybir.ActivationFunctionType.Ln,
                             bias=bias7[:], scale=0.5)

        xp = jpool.tile([NPAIR, NPQ], F32)
        nc.scalar.activation(xp[:], pm[:], mybir.ActivationFunctionType.Ln,
                             bias=bias38[:])
        nc.vector.tensor_mul(xp[:], xp[:], pm[:])

        xg = jpool.tile([NPAIR, NPQ], F32)
        nc.scalar.activation(xg[:], gmat[:], mybir.ActivationFunctionType.Ln,
                             bias=bias38[:])
        nc.vector.tensor_mul(xg[:], xg[:], gmat[:])

        nc.vector.tensor_mul(s[:], s[:], m[:])     # s = (p+g) * m
        nc.vector.tensor_add(xp[:], xp[:], xg[:])
        nc.vector.tensor_sub(xp[:], xp[:], s[:])

        red = jpool.tile([NPAIR, 1], F32)
        nc.vector.tensor_reduce(red[:], xp[:], axis=mybir.AxisListType.X,
                                op=mybir.AluOpType.add)
        nc.sync.dma_start(out[:, :], red[:])


_CACHE = {}


def _prep_in_maps(z, z_pos, z_dis, z_pos_dis, rand_idx):
    zf = z.reshape(B, HW, D)
    zpdf = z_pos_dis.reshape(B, HW, NPQ).astype(np.float32, copy=False)
    zposf = z_pos.reshape(B, HW, D).astype(np.float32, copy=False)
    zdf = z_dis.reshape(B, HW, NPQ)

    ridx = rand_idx.astype(np.int64)
    sample_z = np.take_along_axis(zf, ridx[..., None], axis=1)       # (B,3,D)
    sample_z_dis = np.take_along_axis(zdf, ridx[..., None], axis=1)  # (B,3,NPQ)

    in_maps = []
    for c in range(NCORES):
        bs = slice(c * BPC, (c + 1) * BPC)
        # zpt[bi, ck, cl, j] = z_pos[4c+bi, j, 128*ck+cl]
        zpt = np.ascontiguousarray(
            zposf[bs].transpose(0, 2, 1).reshape(BPC, 2, 128, HW))
        # szt[ck, cl, 32*bi+q] = sample_z[4c+bi, q, 128*ck+cl], zero-pad
        sz = sample_z[bs]                                  # (BPC, 3, D)
        szt = np.zeros((2, 128, 128), np.float32)
        szt_q = sz.reshape(BPC * NQ, 2, 128).transpose(1, 2, 0)  # (2,128,12)
        for bi in range(BPC):
            szt[:, :, 32 * bi:32 * bi + NQ] = szt_q[:, :, NQ * bi:NQ * bi + NQ]
        # pmat row 30*bi + i = sample_z_dis[4c+bi, i % 3]
        szd = sample_z_dis[bs]                             # (BPC, 3, NPQ)
        i = np.arange(NQ * NPOS)
        pmatc = np.ascontiguousarray(
            szd[:, i % NQ, :].reshape(NPAIR, NPQ)).astype(np.float32)
        boffs = np.zeros((128, 1), np.float32)
        for bi in range(BPC):
            boffs[32 * bi:32 * bi + NQ, 0] = bi * HW
        in_maps.append({
            "zpt": zpt,
            "zpdis": np.ascontiguousarray(zpdf[bs].reshape(BPC * HW, NPQ)),
            "szt": szt,
            "pmat": pmatc,
            "boffs": boffs,
        })
    return in_maps


def kernel(z, z_pos, z_dis, z_pos_dis, rand_idx):
    if "nc" not in _CACHE:
        _CACHE["nc"] = build_kernel()
    nc = _CACHE["nc"]
    in_maps = _prep_in_maps(z, z_pos, z_dis, z_pos_dis, rand_idx)
    res = run_bass_kernel_spmd(nc, in_maps, core_ids=list(range(NCORES)))
    total = 0.0
    for c in range(NCORES):
        total += float(res.results[c]["out"].astype(np.float64).sum())
    loss = 0.5 * total / (B * NQ * NPOS)
    return np.float32(loss)



# revision 6
# speedup vs baseline: 1.8786x; 1.8786x over previous
"""Trainium2 Bass kernel for nn_JSDPosLoss: JSD loss over top-k retrieved rows.

Contract: kernel(**inputs) takes FULL numpy inputs, returns FULL output (f32
scalar). Data-parallel over batch across 8 NeuronCores (4 batches/core).

v2 design (memory-regime): the dominant cost is streaming z_pos for the
attention matmul. Stream it as fp8e4m3 (4 MiB/core; attention is only used
for top-k ranking and the loss is insensitive to rank flips) and use
DoubleRow fp8 matmuls (full 256-deep contraction per pass). Zero-padded
lhsT tiles place each batch's 3 query rows at distinct PSUM partitions and
fold the 8 column chunks of each batch into partitions too, so per-batch
attention lands as [64, 512] in PSUM with no PSUM->SBUF copies and top-k
passes cost only 512 columns.

Per batch (pipelined; batches 0-2 fully overlap the remaining stream):
  top-8/chunk via max8+max_index -> pack quantized value+index into one f32
  -> tiny f32 PE matmuls fold candidates across partitions into [3, 64]
  -> max8/match_replace/max8 merge to top-10 -> unpack indices (mod 8192)
  -> indirect-DMA gather of z_pos_dis rows -> JSD partial sums via two
  fused tensor_tensor_reduce passes (+ ACT Ln). Host adds sum(xlogy(p,p)).

Pair rows are padded 30->32 per batch so every engine slice starts at a
0/32/64/96 partition base.
"""

import numpy as np
import ml_dtypes

import concourse.bass as bass
import concourse.bacc as bacc
import concourse.mybir as mybir
import concourse.tile as tile
from concourse.bass_utils import run_bass_kernel_spmd

# Problem dims (hardcoded per contract)
B, H, W, D, NPQ = 32, 64, 64, 256, 512
HW = H * W                  # 4096
NQ, NPOS = 3, 10
NCORES = 8
BPC = B // NCORES           # 4 batches per core
NPR = 32                    # padded pair-rows per batch (30 used)
NPAD = BPC * NPR            # 128 padded pair rows per core

F32 = mybir.dt.float32
F8 = mybir.dt.float8e4
U32 = mybir.dt.uint32
AF = mybir.ActivationFunctionType
ALU = mybir.AluOpType

NCH = 8                     # column chunks per batch row
CW = HW // NCH              # 512 columns per chunk
MAGIC = 12582912.0          # 1.5 * 2**23: float32 round-to-int trick
QS = 16.0                   # value quantization scale for packing
PKS = 8192.0                # index field size in packed floats


def build_kernel():
    nc = bacc.Bacc("TRN2", target_bir_lowering=False, debug=False,
                   num_devices=NCORES)

    # z_pos fp8, DoubleRow layout: [bi, c(part), kt, j]; d = 128*kt + c
    zpt = nc.dram_tensor("zpt", [BPC, 128, 2, HW], F8,
                         kind="ExternalInput").ap()
    # zero-padded stationary tiles [c, bi, v, kt, m] (queries at m=3v+q)
    lw = nc.dram_tensor("lw", [128, BPC, 4, 2, 32], F8,
                        kind="ExternalInput").ap()
    # gather table (f32 rows of z_pos_dis, flattened per core)
    zpdis = nc.dram_tensor("zpdis", [BPC * HW, NPQ], F32,
                           kind="ExternalInput").ap()
    # P matrix rows (sample_z_dis broadcast per torch quirk), padded to 32/batch
    pmat = nc.dram_tensor("pmat", [NPAD, NPQ], F32, kind="ExternalInput").ap()
    # fold selectors [64, 3*NCH] f32: column 3*ch+q has 1 at partition p(ch,q)
    selc = nc.dram_tensor("selc", [64, 3 * NCH], F32,
                          kind="ExternalInput").ap()
    # per-partition chunk column offsets [64, 1] f32 (512 * chunk(p))
    offc = nc.dram_tensor("offc", [64, 1], F32, kind="ExternalInput").ap()
    # outputs per padded pair-row: [sum(g*ln g), sum(s*ln(s/2))]
    out = nc.dram_tensor("out", [NPAD, 2], F32, kind="ExternalOutput").ap()

    with tile.TileContext(nc) as tc:
        _body(tc, nc, zpt, lw, zpdis, pmat, selc, offc, out)
    nc.compile()
    return nc


def _body(tc, nc, zpt, lw, zpdis, pmat, selc, offc, out):
    with (
        tc.tile_pool(name="const", bufs=1) as cpool,
        tc.tile_pool(name="load", bufs=2) as lpool,
        tc.tile_pool(name="attn", bufs=2, space="PSUM") as apool,
        tc.tile_pool(name="fold", bufs=2, space="PSUM") as fpool,
        tc.tile_pool(name="small", bufs=2) as spool,
        tc.tile_pool(name="jsd", bufs=1) as jpool,
    ):
        # ---- constants / startup loads ----
        lw_sb = cpool.tile([128, BPC, 4, 2, 32], F8)
        nc.sync.dma_start(lw_sb[:], lw[:])
        sel_sb = cpool.tile([64, 3 * NCH], F32)
        nc.sync.dma_start(sel_sb[:], selc[:, :])
        off_sb = cpool.tile([64, 1], F32)
        nc.sync.dma_start(off_sb[:], offc[:, :])
        pm = jpool.tile([NPAD, NPQ], F32)
        nc.scalar.dma_start(pm[:], pmat[:, :])

        gidx = cpool.tile([NPAD, 1], U32)        # gather indices
        gmat = jpool.tile([NPAD, NPQ], F32)      # gathered z_pos_dis rows
        acc = cpool.tile([NPAD, 2], F32)         # [sum(g ln g), sum(s m)]
        nc.vector.memset(acc[:], 0.0)
        bias38 = cpool.tile([NPAD, 1], F32)
        nc.vector.memset(bias38[:], 1e-38)
        bias0 = cpool.tile([NPAD, 1], F32)
        nc.vector.memset(bias0[:], 0.0)

        dma_engines = [nc.sync, nc.scalar]
        qi = 0

        for bi in range(BPC):
            # ---- stream this batch's z_pos (fp8) and run attention ----
            lds = []
            for t in range(4):
                ld = lpool.tile([128, 2, 1024], F8, tag=f"ld{t}")
                eng = dma_engines[qi % len(dma_engines)]
                qi += 1
                eng.dma_start(ld[:], zpt[bi, :, :, t * 1024:(t + 1) * 1024])
                lds.append(ld)

            # PSUM [64, 512]: partition p = 32*(ch//4) + 3*(ch%4) + q
            at = apool.tile([64, CW], F32, tag="at")
            for ch in range(NCH):
                blk, v = ch // 4, ch % 4
                rhs = lds[ch // 2][:, :, (ch % 2) * 512:(ch % 2) * 512 + 512]
                nc.tensor.matmul(
                    at[32 * blk:32 * blk + 32, :],
                    lhsT=lw_sb[:, bi, v],
                    rhs=rhs,
                    start=(v == 0), stop=(v == 3),
                    perf_mode=mybir.MatmulPerfMode.DoubleRow)

            # ---- top-8 per chunk (value + index), one pass each ----
            cand = spool.tile([64, 8], F32, tag="cand")
            candi = spool.tile([64, 8], U32, tag="candi")
            nc.vector.max(cand[:], at[:])
            nc.vector.max_index(candi[:], cand[:], at[:])

            # ---- pack quantized value + in-batch column index ----
            idxf = spool.tile([64, 8], F32, tag="idxf")
            nc.vector.tensor_copy(idxf[:], candi[:])
            nc.vector.tensor_scalar(idxf[:], idxf[:], off_sb[:], None,
                                    op0=ALU.add)
            pk = spool.tile([64, 8], F32, tag="pk")
            nc.vector.tensor_scalar(pk[:], cand[:], QS, MAGIC,
                                    op0=ALU.mult, op1=ALU.add)
            nc.vector.tensor_scalar(pk[:], pk[:], MAGIC, PKS,
                                    op0=ALU.subtract, op1=ALU.mult)
            nc.vector.tensor_add(pk[:], pk[:], idxf[:])

            # ---- fold candidates across partitions: [64, 8] -> [3, 64] ----
            fp = fpool.tile([NQ, 8 * NCH], F32, tag="fp")
            for ch in range(NCH):
                nc.tensor.matmul(
                    fp[:, 8 * ch:8 * ch + 8],
                    lhsT=sel_sb[:, 3 * ch:3 * ch + NQ],
                    rhs=pk[:],
                    start=True, stop=True)

            # ---- merge to top-10 packed per row ----
            m1 = spool.tile([NQ, 8], F32, tag="m1")
            nc.vector.max(m1[:], fp[:])
            tmp = spool.tile([NQ, 8 * NCH], F32, tag="tmp")
            nc.vector.match_replace(tmp[:], in_to_replace=m1[:],
                                    in_values=fp[:], imm_value=-1e30)
            m2 = spool.tile([NQ, 8], F32, tag="m2")
            nc.vector.max(m2[:], tmp[:])
            t10 = spool.tile([NQ, NPOS], F32, tag="t10")
            nc.vector.tensor_copy(t10[:, 0:8], m1[:])
            nc.vector.tensor_copy(t10[:, 8:NPOS], m2[:, 0:2])

            # ---- unpack row index; add batch base; to u32 ----
            nc.vector.tensor_scalar(t10[:], t10[:], PKS, float(bi * HW),
                                    op0=ALU.mod, op1=ALU.add)
            i10 = spool.tile([NQ, NPOS], U32, tag="i10")
            nc.vector.tensor_copy(i10[:], t10[:])

            # ---- flatten [3, 10] -> [30, 1] and gather ----
            rj = slice(NPR * bi, NPR * bi + NQ * NPOS)
            nc.sync.dma_start(gidx[rj, :], i10[:])
            nc.gpsimd.indirect_dma_start(
                out=gmat[rj, :], out_offset=None,
                in_=zpdis[:, :],
                in_offset=bass.IndirectOffsetOnAxis(ap=gidx[rj, :1], axis=0))

            # ---- JSD partial sums for this batch's 30 pair-rows ----
            s = jpool.tile([NPAD, NPQ], F32, tag="s")
            nc.vector.tensor_add(s[rj], pm[rj], gmat[rj])
            xg = jpool.tile([NPAD, NPQ], F32, tag="xg")
            nc.scalar.activation(xg[rj], gmat[rj], AF.Ln, bias=bias38[rj])
            m = jpool.tile([NPAD, NPQ], F32, tag="m")
            nc.scalar.activation(m[rj], s[rj], AF.Ln, bias=bias0[rj], scale=0.5)
            tt = jpool.tile([NPAD, NPQ], F32, tag="tt")
            nc.vector.tensor_tensor_reduce(
                out=tt[rj], in0=xg[rj], in1=gmat[rj], scale=1.0, scalar=0.0,
                op0=ALU.mult, op1=ALU.add, accum_out=acc[rj, 0:1])
            nc.vector.tensor_tensor_reduce(
                out=tt[rj], in0=s[rj], in1=m[rj], scale=1.0, scalar=0.0,
                op0=ALU.mult, op1=ALU.add, accum_out=acc[rj, 1:2])

        nc.sync.dma_start(out[:, :], acc[:])


_CACHE = {}


def _prep_in_maps(z, z_pos, z_dis, z_pos_dis, rand_idx):
    f8 = ml_dtypes.float8_e4m3
    zf = z.reshape(B, HW, D)
    zpdf = z_pos_dis.reshape(B, HW, NPQ).astype(np.float32, copy=False)
    zposf = z_pos.reshape(B, HW, D).astype(np.float32, copy=False)
    zdf = z_dis.reshape(B, HW, NPQ)

    ridx = rand_idx.astype(np.int64)
    sample_z = np.take_along_axis(zf, ridx[..., None], axis=1)       # (B,3,D)
    sample_z_dis = np.take_along_axis(zdf, ridx[..., None], axis=1)  # (B,3,NPQ)

    # fold selectors / chunk offsets (shared across cores)
    selc = np.zeros((64, 3 * NCH), np.float32)
    offc = np.zeros((64, 1), np.float32)
    for ch in range(NCH):
        for q in range(NQ):
            p = 32 * (ch // 4) + 3 * (ch % 4) + q
            selc[p, 3 * ch + q] = 1.0
            offc[p, 0] = CW * ch

    jmod = np.arange(NQ * NPOS) % NQ

    in_maps = []
    for c in range(NCORES):
        bs = slice(c * BPC, (c + 1) * BPC)
        # zpt[bi, c, kt, j] = z_pos[4core+bi, j, 128*kt+c]
        zpt = np.ascontiguousarray(
            zposf[bs].transpose(0, 2, 1).reshape(BPC, 2, 128, HW)
            .transpose(0, 2, 1, 3)).astype(f8)
        # lw[c, bi, v, kt, m]: batch bi queries at m = 3v+q
        sz8 = sample_z[bs].astype(f8)                      # (BPC, 3, D)
        szt = np.ascontiguousarray(
            sz8.reshape(BPC, NQ, 2, 128).transpose(3, 0, 2, 1))  # c,bi,kt,q
        lwf = np.zeros((128, BPC, 4, 2, 32), f8)
        for v in range(4):
            lwf[:, :, v, :, 3 * v:3 * v + NQ] = szt
        # pmat padded row 32*bi + j = sample_z_dis[4core+bi, j % 3], j < 30
        szd = sample_z_dis[bs].astype(np.float32)          # (BPC, 3, NPQ)
        pmatc = np.zeros((BPC, NPR, NPQ), np.float32)
        pmatc[:, :NQ * NPOS] = szd[:, jmod, :]
        in_maps.append({
            "zpt": zpt,
            "lw": lwf,
            "zpdis": np.ascontiguousarray(zpdf[bs].reshape(BPC * HW, NPQ)),
            "pmat": pmatc.reshape(NPAD, NPQ),
            "selc": selc,
            "offc": offc,
        })
    return in_maps


def kernel(z, z_pos, z_dis, z_pos_dis, rand_idx):
    if "nc" not in _CACHE:
        _CACHE["nc"] = build_kernel()
    nc = _CACHE["nc"]
    in_maps = _prep_in_maps(z, z_pos, z_dis, z_pos_dis, rand_idx)
    res = run_bass_kernel_spmd(nc, in_maps, core_ids=list(range(NCORES)))

    # host: sum(xlogy(p,p)) + per-row accumulator combine; skip pad rows
    valid = (np.arange(NPAD) % NPR) < NQ * NPOS
    total = 0.0
    for c in range(NCORES):
        o = res.results[c]["out"].astype(np.float64)[valid]
        total += o[:, 0].sum() - o[:, 1].sum()
        p = in_maps[c]["pmat"].astype(np.float64)[valid]
        total += np.where(p > 0, p * np.log(np.where(p > 0, p, 1.0)), 0.0).sum()
    loss = 0.5 * total / (B * NQ * NPOS)
    return np.float32(loss)


# revision 8
# speedup vs baseline: 2.1614x; 1.1505x over previous
"""Trainium2 Bass kernel for nn_JSDPosLoss: JSD loss over top-k retrieved rows.

Contract: kernel(**inputs) takes FULL numpy inputs, returns FULL output (f32
scalar). Data-parallel over batch across 8 NeuronCores (4 batches/core).

v3 design (memory-regime): stream z_pos as fp8e4m3 (4 MiB/core; attention
only ranks top-k and the loss is insensitive to rank flips) into DoubleRow
fp8 matmuls (256-deep contraction per pass). Zero-padded lhsT tiles place
each batch's 3 query rows and all 8 column chunks at distinct partitions of
one [64, 512] PSUM tile, so top-k passes cost only 512 columns and there
are no PSUM->SBUF copies.

Per batch, software-pipelined against the remaining stream:
  max8 + max_index (one 512-col pass each) -> pack quantized value + column
  index into one f32 -> tiny f32 PE matmuls fold candidates across
  partitions into [3, 64] -> max8/match_replace/max8 merge to top-10 ->
  unpack indices (mod 8192) -> indirect gather of bf16 z_pos_dis rows with
  a 2D [3, 10] offset AP (no flatten DMA) -> JSD partials via one fused
  1024-column tensor_tensor_reduce: sum(g*ln g) + sum((-s)*ln(-s*-0.5))
  with -s built from host-negated P rows on the Pool engine. Host adds
  sum(xlogy(p, p)).

Emission is staged (stream/attn/select/jsd interleaved across batches) so
no engine head-blocks on a later dependency; the Ln activation table is
preloaded at t=0.
"""

import numpy as np
import ml_dtypes

import concourse.bass as bass
import concourse.bacc as bacc
import concourse.mybir as mybir
import concourse.tile as tile
from concourse.bass_utils import run_bass_kernel_spmd

# Problem dims (hardcoded per contract)
B, H, W, D, NPQ = 32, 64, 64, 256, 512
HW = H * W                  # 4096
NQ, NPOS = 3, 10
NCORES = 8
BPC = B // NCORES           # 4 batches per core
NPR = 32                    # padded pair-rows per batch (30 used)
NPAD = BPC * NPR            # 128 padded pair rows per core

F32 = mybir.dt.float32
BF16 = mybir.dt.bfloat16
F8 = mybir.dt.float8e4
U32 = mybir.dt.uint32
AF = mybir.ActivationFunctionType
ALU = mybir.AluOpType
JDT = BF16                  # JSD elementwise dtype (accumulation is f32)

NCH = 8                     # column chunks per batch row
CW = HW // NCH              # 512 columns per chunk
MAGIC = 12582912.0          # 1.5 * 2**23: float32 round-to-int trick
QS = 16.0                   # value quantization scale for packing
PKS = 8192.0                # index field size in packed floats


def build_kernel():
    nc = bacc.Bacc("TRN2", target_bir_lowering=False, debug=False,
                   num_devices=NCORES)

    # z_pos fp8, DoubleRow layout: [bi, c(part), kt, j]; d = 128*kt + c
    zpt = nc.dram_tensor("zpt", [BPC, 128, 2, HW], F8,
                         kind="ExternalInput").ap()
    # zero-padded stationary tiles [c, bi, v, kt, m] (queries at m=3v+q)
    lw = nc.dram_tensor("lw", [128, BPC, 4, 2, 32], F8,
                        kind="ExternalInput").ap()
    # gather table (bf16 rows of z_pos_dis, flattened per core)
    zpdis = nc.dram_tensor("zpdis", [BPC * HW, NPQ], BF16,
                           kind="ExternalInput").ap()
    # negated P rows (-sample_z_dis broadcast, torch-quirk order), padded
    pmn = nc.dram_tensor("pmn", [NPAD, NPQ], BF16, kind="ExternalInput").ap()
    # fold selectors [64, 3*NCH] f32 + per-partition chunk column offsets
    selc = nc.dram_tensor("selc", [64, 3 * NCH], F32,
                          kind="ExternalInput").ap()
    offc = nc.dram_tensor("offc", [64, 1], F32, kind="ExternalInput").ap()
    # output per padded pair-row: sum(g ln g) - sum(s ln(s/2))
    out = nc.dram_tensor("out", [NPAD, 1], F32, kind="ExternalOutput").ap()

    with tile.TileContext(nc) as tc:
        _body(tc, nc, zpt, lw, zpdis, pmn, selc, offc, out)
    nc.compile()
    return nc


def _body(tc, nc, zpt, lw, zpdis, pmn, selc, offc, out):
    with (
        tc.tile_pool(name="const", bufs=1) as cpool,
        tc.tile_pool(name="load", bufs=2) as lpool,
        tc.tile_pool(name="attn", bufs=2, space="PSUM") as apool,
        tc.tile_pool(name="fold", bufs=2, space="PSUM") as fpool,
        tc.tile_pool(name="small", bufs=2) as spool,
    ):
        # ---- constants / startup ----
        lw_sb = cpool.tile([128, BPC, 4, 2, 32], F8)
        nc.sync.dma_start(lw_sb[:], lw[:])
        sel_sb = cpool.tile([64, 3 * NCH], F32)
        nc.gpsimd.dma_start(sel_sb[:], selc[:, :])
        off_sb = cpool.tile([64, 1], F32)
        nc.gpsimd.dma_start(off_sb[:], offc[:, :])
        pm = cpool.tile([NPAD, NPQ], JDT)
        nc.scalar.dma_start(pm[:], pmn[:, :])

        gs = cpool.tile([NPAD, 2 * NPQ], JDT)    # [g | -s]
        xgm = cpool.tile([NPAD, 2 * NPQ], JDT)   # [ln(g) | ln(s/2)]
        tt = cpool.tile([NPAD, 2 * NPQ], JDT)    # ttr elementwise scratch
        acc = cpool.tile([NPAD, 1], F32)
        nc.vector.memset(acc[:], 0.0)
        one = cpool.tile([32, 1], F32)
        nc.vector.memset(one[:], 1.0)
        bias38 = cpool.tile([NPAD, 1], F32)
        nc.vector.memset(bias38[:], 1e-38)
        bias0 = cpool.tile([NPAD, 1], F32)
        nc.vector.memset(bias0[:], 0.0)
        # preload the Ln activation table off the critical path
        nc.scalar.activation(one[:], one[:], AF.Ln, bias=bias0[0:32])

        lds = {}
        i10s = {}

        def stage_stream(bi):
            for t in range(2):
                ld = lpool.tile([128, 2, 2048], F8, tag=f"ld{t}")
                eng = nc.sync if t == 0 else nc.scalar
                eng.dma_start(ld[:], zpt[bi, :, :, t * 2048:(t + 1) * 2048])
                lds[(bi, t)] = ld

        def stage_attn(bi):
            # PSUM [64, 512]: partition p = 32*(ch//4) + 3*(ch%4) + q
            at = apool.tile([64, CW], F32, tag="at")
            for ch in range(NCH):
                blk, v = ch // 4, ch % 4
                rhs = lds[(bi, ch // 4)][:, :, (ch % 4) * 512:
                                         (ch % 4) * 512 + 512]
                nc.tensor.matmul(
                    at[32 * blk:32 * blk + 32, :],
                    lhsT=lw_sb[:, bi, v],
                    rhs=rhs,
                    start=(v == 0), stop=(v == 3),
                    perf_mode=mybir.MatmulPerfMode.DoubleRow)
            return at

        def stage_select(bi, at):
            # top-8 per chunk (value + index), one 512-col pass each
            cand = spool.tile([64, 8], F32, tag="cand")
            candi = spool.tile([64, 8], U32, tag="candi")
            nc.vector.max(cand[:], at[:])
            nc.vector.max_index(candi[:], cand[:], at[:])

            # pack quantized value + in-batch column index into one f32
            idxf = spool.tile([64, 8], F32, tag="idxf")
            nc.vector.tensor_scalar(idxf[:], candi[:], off_sb[:], None,
                                    op0=ALU.add)
            pk = spool.tile([64, 8], F32, tag="pk")
            nc.vector.tensor_scalar(pk[:], cand[:], QS, MAGIC,
                                    op0=ALU.mult, op1=ALU.add)
            nc.vector.tensor_scalar(pk[:], pk[:], PKS, MAGIC * PKS,
                                    op0=ALU.mult, op1=ALU.subtract)
            nc.vector.tensor_add(pk[:], pk[:], idxf[:])

            # fold candidates across partitions: [64, 8] -> [3, 64]
            fp = fpool.tile([NQ, 8 * NCH], F32, tag="fp")
            for ch in range(NCH):
                nc.tensor.matmul(
                    fp[:, 8 * ch:8 * ch + 8],
                    lhsT=sel_sb[:, 3 * ch:3 * ch + NQ],
                    rhs=pk[:],
                    start=True, stop=True)

            # merge to top-10 packed per row
            m1 = spool.tile([NQ, 8], F32, tag="m1")
            nc.vector.max(m1[:], fp[:])
            tmp = spool.tile([NQ, 8 * NCH], F32, tag="tmp")
            nc.vector.match_replace(tmp[:], in_to_replace=m1[:],
                                    in_values=fp[:], imm_value=-1e30)
            m2 = spool.tile([NQ, 8], F32, tag="m2")
            nc.vector.max(m2[:], tmp[:])

            # unpack row index (+ batch base) straight to u32, [3, 10]
            i10 = spool.tile([NQ, NPOS], U32, tag="i10")
            nc.vector.tensor_scalar(i10[:, 0:8], m1[:], PKS, float(bi * HW),
                                    op0=ALU.mod, op1=ALU.add)
            nc.vector.tensor_scalar(i10[:, 8:NPOS], m2[:, 0:2], PKS,
                                    float(bi * HW),
                                    op0=ALU.mod, op1=ALU.add)
            i10s[bi] = i10

            # gather the selected bf16 z_pos_dis rows: offsets given 2D
            rj = slice(NPR * bi, NPR * bi + NQ * NPOS)
            nc.gpsimd.indirect_dma_start(
                out=gs[rj, 0:NPQ], out_offset=None,
                in_=zpdis[:, :],
                in_offset=bass.IndirectOffsetOnAxis(ap=i10[:, :], axis=0))

        def stage_sub(bi):
            # -s = (-p) - g on the Pool engine
            rj = slice(NPR * bi, NPR * bi + NQ * NPOS)
            nc.gpsimd.tensor_sub(gs[rj, NPQ:], pm[rj], gs[rj, 0:NPQ])

        def stage_ln(bi):
            rj = slice(NPR * bi, NPR * bi + NQ * NPOS)
            nc.scalar.activation(xgm[rj, 0:NPQ], gs[rj, 0:NPQ], AF.Ln,
                                 bias=bias38[rj])
            nc.scalar.activation(xgm[rj, NPQ:], gs[rj, NPQ:], AF.Ln,
                                 bias=bias0[rj], scale=-0.5)

        def stage_ttr(bi):
            # acc = sum(g*ln g) + sum((-s)*ln(s/2)) over 1024 columns
            rj = slice(NPR * bi, NPR * bi + NQ * NPOS)
            nc.vector.tensor_tensor_reduce(
                out=tt[rj], in0=gs[rj], in1=xgm[rj], scale=1.0, scalar=0.0,
                op0=ALU.mult, op1=ALU.add, accum_out=acc[rj])

        # ---- software-pipelined emission ----
        stage_stream(0)
        stage_stream(1)
        at0 = stage_attn(0)
        at1 = stage_attn(1)
        stage_select(0, at0)
        stage_stream(2)
        at2 = stage_attn(2)
        stage_select(1, at1)
        stage_sub(0)
        stage_ln(0)
        stage_ttr(0)
        stage_stream(3)
        at3 = stage_attn(3)
        stage_select(2, at2)
        stage_sub(1)
        stage_ln(1)
        stage_ttr(1)
        stage_select(3, at3)
        stage_sub(2)
        stage_ln(2)
        stage_ttr(2)
        stage_sub(3)
        stage_ln(3)
        stage_ttr(3)

        nc.sync.dma_start(out[:, :], acc[:])


_CACHE = {}


def _prep_in_maps(z, z_pos, z_dis, z_pos_dis, rand_idx):
    f8 = ml_dtypes.float8_e4m3
    bf = ml_dtypes.bfloat16
    zf = z.reshape(B, HW, D)
    zpdf = z_pos_dis.reshape(B, HW, NPQ).astype(np.float32, copy=False)
    zposf = z_pos.reshape(B, HW, D).astype(np.float32, copy=False)
    zdf = z_dis.reshape(B, HW, NPQ)

    ridx = rand_idx.astype(np.int64)
    sample_z = np.take_along_axis(zf, ridx[..., None], axis=1)       # (B,3,D)
    sample_z_dis = np.take_along_axis(zdf, ridx[..., None], axis=1)  # (B,3,NPQ)

    # fold selectors / chunk offsets (shared across cores)
    selc = np.zeros((64, 3 * NCH), np.float32)
    offc = np.zeros((64, 1), np.float32)
    for ch in range(NCH):
        for q in range(NQ):
            p = 32 * (ch // 4) + 3 * (ch % 4) + q
            selc[p, 3 * ch + q] = 1.0
            offc[p, 0] = CW * ch

    jmod = np.arange(NQ * NPOS) % NQ

    in_maps = []
    for c in range(NCORES):
        bs = slice(c * BPC, (c + 1) * BPC)
        # zpt[bi, c, kt, j] = z_pos[4core+bi, j, 128*kt+c]
        zpt = np.ascontiguousarray(
            zposf[bs].transpose(0, 2, 1).reshape(BPC, 2, 128, HW)
            .transpose(0, 2, 1, 3)).astype(f8)
        # lw[c, bi, v, kt, m]: batch bi queries at m = 3v+q
        sz8 = sample_z[bs].astype(f8)                      # (BPC, 3, D)
        szt = np.ascontiguousarray(
            sz8.reshape(BPC, NQ, 2, 128).transpose(3, 0, 2, 1))  # c,bi,kt,q
        lwf = np.zeros((128, BPC, 4, 2, 32), f8)
        for v in range(4):
            lwf[:, :, v, :, 3 * v:3 * v + NQ] = szt
        # negated P rows, padded: row 32*bi + j = -sample_z_dis[., j % 3]
        szd = sample_z_dis[bs].astype(np.float32)          # (BPC, 3, NPQ)
        pmn = np.zeros((BPC, NPR, NPQ), np.float32)
        pmn[:, :NQ * NPOS] = -szd[:, jmod, :]
        in_maps.append({
            "zpt": zpt,
            "lw": lwf,
            "zpdis": np.ascontiguousarray(
                zpdf[bs].reshape(BPC * HW, NPQ)).astype(bf),
            "pmn": pmn.reshape(NPAD, NPQ).astype(bf),
            "selc": selc,
            "offc": offc,
        })
    return in_maps


def kernel(z, z_pos, z_dis, z_pos_dis, rand_idx):
    if "nc" not in _CACHE:
        _CACHE["nc"] = build_kernel()
    nc = _CACHE["nc"]
    in_maps = _prep_in_maps(z, z_pos, z_dis, z_pos_dis, rand_idx)
    res = run_bass_kernel_spmd(nc, in_maps, core_ids=list(range(NCORES)))

    # host: sum(xlogy(p,p)) + per-row accumulator; skip pad rows
    valid = (np.arange(NPAD) % NPR) < NQ * NPOS
    total = 0.0
    for c in range(NCORES):
        o = res.results[c]["out"].astype(np.float64)[valid]
        total += o[:, 0].sum()
        p = -in_maps[c]["pmn"].astype(np.float64)[valid]
        total += np.where(p > 0, p * np.log(np.where(p > 0, p, 1.0)), 0.0).sum()
    loss = 0.5 * total / (B * NQ * NPOS)
    return np.float32(loss)


# revision 11
# speedup vs baseline: 2.4792x; 1.1470x over previous
"""Trainium2 Bass kernel for nn_JSDPosLoss: JSD loss over top-k retrieved rows.

Contract: kernel(**inputs) takes FULL numpy inputs, returns FULL output (f32
scalar). Data-parallel over batch across 8 NeuronCores (4 batches/core).

v3 design (memory-regime): stream z_pos as fp8e4m3 (4 MiB/core; attention
only ranks top-k and the loss is insensitive to rank flips) into DoubleRow
fp8 matmuls (256-deep contraction per pass). Zero-padded lhsT tiles place
each batch's 3 query rows and all 8 column chunks at distinct partitions of
one [64, 512] PSUM tile, so top-k passes cost only 512 columns and there
are no PSUM->SBUF copies.

Per batch, software-pipelined against the remaining stream:
  max8 + max_index (one 512-col pass each) -> pack quantized value + column
  index into one f32 -> tiny f32 PE matmuls fold candidates across
  partitions into [3, 64] -> max8/match_replace/max8 merge to top-10 ->
  unpack indices (mod 8192) -> indirect gather of bf16 z_pos_dis rows with
  a 2D [3, 10] offset AP (no flatten DMA) -> JSD partials via one fused
  1024-column tensor_tensor_reduce: sum(g*ln g) + sum((-s)*ln(-s*-0.5))
  with -s built from host-negated P rows on the Pool engine. Host adds
  sum(xlogy(p, p)).

Emission is staged (stream/attn/select/jsd interleaved across batches) so
no engine head-blocks on a later dependency; the Ln activation table is
preloaded at t=0.
"""

import numpy as np
import ml_dtypes

import concourse.bass as bass
import concourse.bacc as bacc
import concourse.mybir as mybir
import concourse.tile as tile
from concourse.bass_utils import run_bass_kernel_spmd

# Problem dims (hardcoded per contract)
B, H, W, D, NPQ = 32, 64, 64, 256, 512
HW = H * W                  # 4096
NQ, NPOS = 3, 10
NCORES = 8
BPC = B // NCORES           # 4 batches per core
NPR = 32                    # padded pair-rows per batch (30 used)
NPAD = BPC * NPR            # 128 padded pair rows per core

F32 = mybir.dt.float32
BF16 = mybir.dt.bfloat16
F8 = mybir.dt.float8e4
U32 = mybir.dt.uint32
AF = mybir.ActivationFunctionType
ALU = mybir.AluOpType
JDT = BF16                  # JSD elementwise dtype (accumulation is f32)

NCH = 8                     # column chunks per batch row
CW = HW // NCH              # 512 columns per chunk
MAGIC = 12582912.0          # 1.5 * 2**23: float32 round-to-int trick
QS = 16.0                   # value quantization scale for packing
PKS = 8192.0                # index field size in packed floats


def build_kernel():
    nc = bacc.Bacc("TRN2", target_bir_lowering=False, debug=False,
                   num_devices=NCORES)

    # z_pos fp8, DoubleRow layout: [bi, c(part), kt, j]; d = 128*kt + c
    zpt = nc.dram_tensor("zpt", [BPC, 128, 2, HW], F8,
                         kind="ExternalInput").ap()
    # zero-padded stationary tiles [c, bi, v, kt, m] (queries at m=3v+q)
    lw = nc.dram_tensor("lw", [128, BPC, 4, 2, 32], F8,
                        kind="ExternalInput").ap()
    # gather table (bf16 rows of z_pos_dis, flattened per core)
    zpdis = nc.dram_tensor("zpdis", [BPC * HW, NPQ], BF16,
                           kind="ExternalInput").ap()
    # negated P rows (-sample_z_dis broadcast, torch-quirk order), padded
    pmn = nc.dram_tensor("pmn", [NPAD, NPQ], BF16, kind="ExternalInput").ap()
    # fold selectors [64, 3*NCH] f32 + per-partition chunk column offsets
    selc = nc.dram_tensor("selc", [64, 3 * NCH], F32,
                          kind="ExternalInput").ap()
    offc = nc.dram_tensor("offc", [64, 1], F32, kind="ExternalInput").ap()
    # output per padded pair-row: sum(g ln g) - sum(s ln(s/2))
    out = nc.dram_tensor("out", [NPAD, 1], F32, kind="ExternalOutput").ap()

    with tile.TileContext(nc) as tc:
        _body(tc, nc, zpt, lw, zpdis, pmn, selc, offc, out)
    nc.compile()
    return nc


def _body(tc, nc, zpt, lw, zpdis, pmn, selc, offc, out):
    with (
        tc.tile_pool(name="const", bufs=1) as cpool,
        tc.tile_pool(name="load", bufs=2) as lpool,
        tc.tile_pool(name="attn", bufs=2, space="PSUM") as apool,
        tc.tile_pool(name="fold", bufs=2, space="PSUM") as fpool,
        tc.tile_pool(name="small", bufs=2) as spool,
    ):
        # ---- constants / startup ----
        lw_sb = cpool.tile([128, BPC, 4, 2, 32], F8)
        nc.sync.dma_start(lw_sb[:], lw[:])
        sel_sb = cpool.tile([64, 3 * NCH], F32)
        nc.gpsimd.dma_start(sel_sb[:], selc[:, :])
        off_sb = cpool.tile([64, 1], F32)
        nc.gpsimd.dma_start(off_sb[:], offc[:, :])
        pm = cpool.tile([NPAD, NPQ], JDT)
        nc.scalar.dma_start(pm[:], pmn[:, :])

        gs = cpool.tile([NPAD, 2 * NPQ], JDT)    # [g | -s]
        xgm = cpool.tile([NPAD, 2 * NPQ], JDT)   # [ln(g) | ln(s/2)]
        tt = cpool.tile([NPAD, 2 * NPQ], JDT)    # ttr elementwise scratch
        acc = cpool.tile([NPAD, 1], F32)
        nc.vector.memset(acc[:], 0.0)
        nc.vector.memset(gs[:], 0.0)
        nc.vector.memset(xgm[:], 0.0)
        one = cpool.tile([32, 1], F32)
        nc.vector.memset(one[:], 1.0)
        bias38 = cpool.tile([NPAD, 1], F32)
        nc.vector.memset(bias38[:], 1e-38)

        lds = {}

        # stream segmentation per batch: b0 split for an early pipeline
        # start, b3 quartered so its matmuls drain right behind the stream
        SEGS = {0: 2, 1: 1, 2: 1, 3: 4}

        def stage_stream(bi, engs):
            n = SEGS[bi]
            w = HW // n
            segs = []
            for t in range(n):
                ld = lpool.tile([128, 2, w], F8, tag=f"ld{bi}_{t}")
                engs[t % len(engs)].dma_start(
                    ld[:], zpt[bi, :, :, t * w:(t + 1) * w])
                segs.append(ld)
            lds[bi] = (segs, w)

        def stage_attn(bi):
            # PSUM [64, 512]: partition p = 32*(ch//4) + 3*(ch%4) + q
            segs, w = lds[bi]
            at = apool.tile([64, CW], F32, tag="at")
            for ch in range(NCH):
                blk, v = ch // 4, ch % 4
                base = ch * CW
                rhs = segs[base // w][:, :, base % w:base % w + CW]
                nc.tensor.matmul(
                    at[32 * blk:32 * blk + 32, :],
                    lhsT=lw_sb[:, bi, v],
                    rhs=rhs,
                    start=(v == 0), stop=(v == 3),
                    perf_mode=mybir.MatmulPerfMode.DoubleRow)
            return at

        def stage_select(bi, at):
            # top-8 per chunk (value + index), one 512-col pass each
            cand = spool.tile([64, 8], F32, tag="cand")
            candi = spool.tile([64, 8], U32, tag="candi")
            nc.vector.max(cand[:], at[:])
            nc.vector.max_index(candi[:], cand[:], at[:])

            # pack quantized value + in-batch column index into one f32
            idxf = spool.tile([64, 8], F32, tag="idxf")
            nc.vector.tensor_scalar(idxf[:], candi[:], off_sb[:], None,
                                    op0=ALU.add)
            pk = spool.tile([64, 8], F32, tag="pk")
            nc.vector.tensor_scalar(pk[:], cand[:], QS, MAGIC,
                                    op0=ALU.mult, op1=ALU.add)
            nc.vector.tensor_scalar(pk[:], pk[:], PKS, MAGIC * PKS,
                                    op0=ALU.mult, op1=ALU.subtract)
            nc.vector.tensor_add(pk[:], pk[:], idxf[:])

            # fold candidates across partitions: [64, 8] -> [3, 64]
            fp = fpool.tile([NQ, 8 * NCH], F32, tag="fp")
            for ch in range(NCH):
                nc.tensor.matmul(
                    fp[:, 8 * ch:8 * ch + 8],
                    lhsT=sel_sb[:, 3 * ch:3 * ch + NQ],
                    rhs=pk[:],
                    start=True, stop=True)

            # merge to top-10 packed per row
            m1 = spool.tile([NQ, 8], F32, tag="m1")
            nc.vector.max(m1[:], fp[:])
            tmp = spool.tile([NQ, 8 * NCH], F32, tag="tmp")
            nc.vector.match_replace(tmp[:], in_to_replace=m1[:],
                                    in_values=fp[:], imm_value=-1e30)
            m2 = spool.tile([NQ, 8], F32, tag="m2")
            nc.vector.max(m2[:], tmp[:])

            # unpack row index (+ batch base) straight to u32, [3, 10]
            i10 = spool.tile([NQ, NPOS], U32, tag="i10")
            nc.vector.tensor_scalar(i10[:, 0:8], m1[:], PKS, float(bi * HW),
                                    op0=ALU.mod, op1=ALU.add)
            nc.vector.tensor_scalar(i10[:, 8:NPOS], m2[:, 0:2], PKS,
                                    float(bi * HW),
                                    op0=ALU.mod, op1=ALU.add)
            # gather the selected bf16 z_pos_dis rows: offsets given 2D
            rj = slice(NPR * bi, NPR * bi + NQ * NPOS)
            nc.gpsimd.indirect_dma_start(
                out=gs[rj, 0:NPQ], out_offset=None,
                in_=zpdis[:, :],
                in_offset=bass.IndirectOffsetOnAxis(ap=i10[:, :], axis=0))

        def stage_sub(pair):
            # -s = (-p) - g on the Pool engine, two batches per pass
            rp = slice(64 * pair, 64 * pair + 64)
            nc.gpsimd.tensor_sub(gs[rp, NPQ:], pm[rp], gs[rp, 0:NPQ])

        def stage_ln(pair):
            rp = slice(64 * pair, 64 * pair + 64)
            nc.scalar.activation(xgm[rp, 0:NPQ], gs[rp, 0:NPQ], AF.Ln,
                                 bias=bias38[rp])
            nc.scalar.activation(xgm[rp, NPQ:], gs[rp, NPQ:], AF.Ln,
                                 bias=bias38[rp], scale=-0.5)

        # ---- software-pipelined emission ----
        stage_stream(0, [nc.sync, nc.scalar])
        stage_stream(1, [nc.sync])
        at0 = stage_attn(0)
        at1 = stage_attn(1)
        # preload the Ln activation table off the critical path
        nc.scalar.activation(one[:], one[:], AF.Ln, bias=bias38[0:32])
        stage_select(0, at0)
        stage_stream(2, [nc.scalar])
        at2 = stage_attn(2)
        stage_select(1, at1)
        stage_stream(3, [nc.sync, nc.scalar])
        at3 = stage_attn(3)
        stage_select(2, at2)
        stage_sub(0)
        stage_ln(0)
        stage_select(3, at3)
        stage_sub(1)
        stage_ln(1)

        # acc = sum(g*ln g) + sum((-s)*ln(s/2)) over 1024 cols, all batches
        nc.vector.tensor_tensor_reduce(
            out=tt[:], in0=gs[:], in1=xgm[:], scale=1.0, scalar=0.0,
            op0=ALU.mult, op1=ALU.add, accum_out=acc[:])

        nc.sync.dma_start(out[:, :], acc[:])


_CACHE = {}


def _prep_in_maps(z, z_pos, z_dis, z_pos_dis, rand_idx):
    f8 = ml_dtypes.float8_e4m3
    bf = ml_dtypes.bfloat16
    zf = z.reshape(B, HW, D)
    zpdf = z_pos_dis.reshape(B, HW, NPQ).astype(np.float32, copy=False)
    zposf = z_pos.reshape(B, HW, D).astype(np.float32, copy=False)
    zdf = z_dis.reshape(B, HW, NPQ)

    ridx = rand_idx.astype(np.int64)
    sample_z = np.take_along_axis(zf, ridx[..., None], axis=1)       # (B,3,D)
    sample_z_dis = np.take_along_axis(zdf, ridx[..., None], axis=1)  # (B,3,NPQ)

    # fold selectors / chunk offsets (shared across cores)
    selc = np.zeros((64, 3 * NCH), np.float32)
    offc = np.zeros((64, 1), np.float32)
    for ch in range(NCH):
        for q in range(NQ):
            p = 32 * (ch // 4) + 3 * (ch % 4) + q
            selc[p, 3 * ch + q] = 1.0
            offc[p, 0] = CW * ch

    jmod = np.arange(NQ * NPOS) % NQ

    in_maps = []
    for c in range(NCORES):
        bs = slice(c * BPC, (c + 1) * BPC)
        # zpt[bi, c, kt, j] = z_pos[4core+bi, j, 128*kt+c]
        zpt = np.ascontiguousarray(
            zposf[bs].transpose(0, 2, 1).reshape(BPC, 2, 128, HW)
            .transpose(0, 2, 1, 3)).astype(f8)
        # lw[c, bi, v, kt, m]: batch bi queries at m = 3v+q
        sz8 = sample_z[bs].astype(f8)                      # (BPC, 3, D)
        szt = np.ascontiguousarray(
            sz8.reshape(BPC, NQ, 2, 128).transpose(3, 0, 2, 1))  # c,bi,kt,q
        lwf = np.zeros((128, BPC, 4, 2, 32), f8)
        for v in range(4):
            lwf[:, :, v, :, 3 * v:3 * v + NQ] = szt
        # negated P rows, padded: row 32*bi + j = -sample_z_dis[., j % 3]
        szd = sample_z_dis[bs].astype(np.float32)          # (BPC, 3, NPQ)
        pmn = np.zeros((BPC, NPR, NPQ), np.float32)
        pmn[:, :NQ * NPOS] = -szd[:, jmod, :]
        in_maps.append({
            "zpt": zpt,
            "lw": lwf,
            "zpdis": np.ascontiguousarray(
                zpdf[bs].reshape(BPC * HW, NPQ)).astype(bf),
            "pmn": pmn.reshape(NPAD, NPQ).astype(bf),
            "selc": selc,
            "offc": offc,
        })
    return in_maps


def kernel(z, z_pos, z_dis, z_pos_dis, rand_idx):
    if "nc" not in _CACHE:
        _CACHE["nc"] = build_kernel()
    nc = _CACHE["nc"]
    in_maps = _prep_in_maps(z, z_pos, z_dis, z_pos_dis, rand_idx)
    res = run_bass_kernel_spmd(nc, in_maps, core_ids=list(range(NCORES)))

    # host: sum(xlogy(p,p)) + per-row accumulator; skip pad rows
    valid = (np.arange(NPAD) % NPR) < NQ * NPOS
    total = 0.0
    for c in range(NCORES):
        o = res.results[c]["out"].astype(np.float64)[valid]
        total += o[:, 0].sum()
        p = -in_maps[c]["pmn"].astype(np.float64)[valid]
        total += np.where(p > 0, p * np.log(np.where(p > 0, p, 1.0)), 0.0).sum()
    loss = 0.5 * total / (B * NQ * NPOS)
    return np.float32(loss)
